# revision 26
# baseline (speedup 1.0000x reference)
"""Trainium2 Bass kernel for nn_CenterAwarePseudoModule (retrieval_knn).

Reference (per row i of feats, per centroid j):
    f_i   = [feats_i, 1] / ||[feats_i, 1]||
    d2_ij = ||f_i||^2 + ||c_j||^2 - 2 f_i . c_j
    out_i = labelset[argmin_j sqrt(max(d2_ij, 0))]

argmin_j d2_ij == argmax_j s_ij with
    s_ij = G'_ij + (inv2_i * cb_j - h_j)
where inv2_i = 2 / sqrt(||feats_i||^2 + 1) (HOST-computed),
G' = (feats * inv2) @ initc[:, :D].T (feats pre-scaled on host),
cb_j = initc[j, D], h_j = ||c_j||^2. Per-row positive affine transforms
preserve the argmin; sqrt/clamp are monotone.

Structure (per core; rows data-parallel over 8 cores):
  - G' on the PE: stationary = ft k-tile [128, 128 rows] fp16, moving =
    ct k-tile halves [128, 500] fp16. fp16 keeps DMA at 2 B/elem (the
    prologue is DMA-bound) and its 11-bit mantissa keeps the argmax
    exact on this data (host-sim: 0 flips; bf16 had 1 flip -> rel err
    1.8e-2, too close to the 2e-2 gate; fp32 doubles prologue bytes).
  - bias entirely on DVE in fp32: t = cbb*inv2_m - hb precomputed per
    m-tile during the PE sweep (scalar_tensor_tensor, [128,1] scalar),
    epilogue s = psum + t, then top-8 max + max_index into a
    [128, MT, 8] staging tile; ONE output DMA at the end (16 scattered
    [128,1] column DMAs cost ~7us of completion-wait in the tail).
  - DMA issue is the ramp bottleneck: each DMA_DIRECT2D costs ~0.8us
    of serial engine time and nothing issues before a ~6.6us program
    preamble. So issue on BOTH hardware-DGE engines in parallel:
    ct + consts on the (otherwise idle) scalar engine, ft on sync.
  - ramp: first 4 m-tiles run k-outer (4 m x 2 cent-halves = 8 PSUM
    banks) so PE consumption tracks the ct stream; everything is
    SBUF-resident in fp16 (ct 32KB + ft 64KB per partition).
"""
import sys

sys.path.insert(0, "/opt/trn_rl_repo")

import numpy as np

N, D, NCENT = 16384, 2048, 1000
NCORES = 8
R = N // NCORES          # rows per core = 2048
MT = R // 128            # m-tiles per core = 16
KT = D // 128            # contraction tiles = 16
NH = 500                 # centroid half (fits one PSUM bank: 500 fp32)
W = 4                    # phase-1 m-tile window (W*2 PSUM banks)

DTF_NAME = "float16"     # PE input dtype: float16 | bfloat16 | float32r

_cache = {}


def _np_dtf():
    if DTF_NAME == "float16":
        return np.float16
    if DTF_NAME == "bfloat16":
        import ml_dtypes
        return ml_dtypes.bfloat16
    return np.float32


def _build():
    import concourse.bacc as bacc
    import concourse.tile as tile
    from concourse import mybir

    dt = mybir.dt
    dtf = getattr(dt, DTF_NAME)

    nc = bacc.Bacc("TRN2", target_bir_lowering=False, debug=False)

    ft = nc.dram_tensor("ft", [MT, 128, KT, 128], dtf, kind="ExternalInput")
    ct = nc.dram_tensor("ct", [128, KT, NCENT], dtf, kind="ExternalInput")
    cbb = nc.dram_tensor("cbb", [128, NCENT], dt.float32, kind="ExternalInput")
    hbb = nc.dram_tensor("hbb", [128, NCENT], dt.float32, kind="ExternalInput")
    ivc = nc.dram_tensor("ivc", [128, MT], dt.float32, kind="ExternalInput")
    outp = nc.dram_tensor("pred", [128, MT, 8], dt.uint32, kind="ExternalOutput")

    with tile.TileContext(nc) as tc:
        with (
            tc.tile_pool(name="const", bufs=1) as constp,
            tc.tile_pool(name="epi", bufs=2) as epi,
            tc.tile_pool(name="tp", bufs=5) as tpool,
            tc.tile_pool(name="psA", bufs=W, space="PSUM") as psa_pool,
            tc.tile_pool(name="psB", bufs=W, space="PSUM") as psb_pool,
        ):
            ivc_sb = constp.tile([128, MT], dt.float32, tag="ivc")
            cbb_sb = constp.tile([128, NCENT], dt.float32, tag="cbb")
            hb_sb = constp.tile([128, NCENT], dt.float32, tag="hb")
            stage = constp.tile([128, MT, 8], dt.uint32, tag="stage")

            # ct lives in 5 grouped tiles so each can land in ONE big
            # DMA; ct_slice(k) picks the k-tile view inside its group.
            CT_GROUPS = [(0, 1), (1, 4), (4, 8), (8, 12), (12, 16)]
            ct_group_tiles = [
                constp.tile([128, hi - lo, NCENT], dtf, tag=f"ctg{lo}",
                            name=f"ctg{lo}")
                for lo, hi in CT_GROUPS
            ]
            ct_group_of = {}
            for gi, (lo, hi) in enumerate(CT_GROUPS):
                for k in range(lo, hi):
                    ct_group_of[k] = (gi, lo)

            def ct_slice(k):
                gi, lo = ct_group_of[k]
                return ct_group_tiles[gi][:, k - lo, :]
            ft_tiles = [
                constp.tile([128, KT, 128], dtf, tag=f"ft{m}", name=f"ftt{m}")
                for m in range(MT)
            ]

            # ---- DMA plan. One ring (parallel rings do NOT add
            # bandwidth: v4 measured 230+115 B/ns split vs 405 single).
            # The ring holds ~8 DMA instructions and recycles their
            # completion semaphores, so many small DMAs throttle on the
            # ~1.5us completion-notification lag, not on bandwidth ->
            # use FEW, BIG DMAs in arrival-priority order. ivc rides
            # the scalar ring (tiny, needed only by the DVE bias). ----
            nc.scalar.dma_start(ivc_sb[:], ivc.ap())

            H = KT // 2

            def dma_ct_group(lo, hi):
                gi = [g for g, (l, h) in enumerate(CT_GROUPS)
                      if l == lo and h == hi][0]
                nc.sync.dma_start(
                    ct_group_tiles[gi][:], ct.ap()[:, lo:hi, :]
                )

            def dma_ft_half(m, half):
                lo, hi = (0, H) if half == 0 else (H, KT)
                nc.sync.dma_start(
                    ft_tiles[m][:, lo:hi, :], ft.ap()[m][:, lo:hi, :]
                )

            # k0's whole column (ct k0 + all four ft first-halves)
            # lands first and consecutively: the PE then runs without a
            # gap from the first matmul, so the HAM clock-gate warms at
            # ~15us instead of being reset by an early stall.
            dma_ct_group(0, 1)
            dma_ft_half(0, 0)
            dma_ft_half(1, 0)
            dma_ft_half(2, 0)
            dma_ft_half(3, 0)
            dma_ct_group(1, 4)
            dma_ct_group(4, 8)
            dma_ft_half(0, 1)
            dma_ft_half(1, 1)
            dma_ft_half(2, 1)
            dma_ft_half(3, 1)
            dma_ct_group(8, 12)
            dma_ct_group(12, 16)
            nc.sync.dma_start(cbb_sb[:], cbb.ap())
            nc.sync.dma_start(hb_sb[:], hbb.ap())
            for m in range(W, MT):
                nc.sync.dma_start(ft_tiles[m][:], ft.ap()[m])

            def k_mms(m, k, psA, psB):
                lhs = ft_tiles[m][:, k, :]
                ctk = ct_slice(k)
                first = k == 0
                last = k == KT - 1
                nc.tensor.matmul(psA[:], lhs, ctk[:, 0:NH],
                                 start=first, stop=last)
                nc.tensor.matmul(psB[:], lhs, ctk[:, NH:NCENT],
                                 start=first, stop=last)

            def t_stts(m):
                # t = cbb * inv2_m - hb   (DVE, runs during the PE sweep)
                t_sb = tpool.tile([128, NCENT], dt.float32, tag="t",
                                  name=f"t{m}")
                for lo, hi in ((0, NH), (NH, NCENT)):
                    nc.vector.scalar_tensor_tensor(
                        out=t_sb[:, lo:hi], in0=cbb_sb[:, lo:hi],
                        scalar=ivc_sb[:, m:m + 1], in1=hb_sb[:, lo:hi],
                        op0=mybir.AluOpType.mult,
                        op1=mybir.AluOpType.subtract,
                    )
                return t_sb

            def epilogue(m, psA, psB, t_sb):
                s_sb = epi.tile([128, NCENT], dt.float32, tag="s",
                                name=f"s{m}")
                nc.vector.scalar_tensor_tensor(
                    out=s_sb[:, 0:NH], in0=psA[:], scalar=1.0,
                    in1=t_sb[:, 0:NH],
                    op0=mybir.AluOpType.mult, op1=mybir.AluOpType.add,
                )
                nc.vector.scalar_tensor_tensor(
                    out=s_sb[:, NH:NCENT], in0=psB[:], scalar=1.0,
                    in1=t_sb[:, NH:NCENT],
                    op0=mybir.AluOpType.mult, op1=mybir.AluOpType.add,
                )
                mx_sb = epi.tile([128, 8], dt.float32, tag="mx",
                                 name=f"mx{m}")
                nc.vector.max(mx_sb[:], s_sb[:])
                nc.vector.max_index(stage[:, m, :], mx_sb[:], s_sb[:])

            # ---- phase 1: junk fp16 warm-up matmuls (the PE HAM
            # clock-gate starts at 1.2 GHz and needs ~3.4us of sustained
            # activity to reach 2.4 GHz; these run while the first ct/ft
            # DMAs stream in, and the real start=True k0 matmuls
            # overwrite the banks), then a k-outer sweep over the first
            # W m-tiles ----
            ps1 = [
                (psa_pool.tile([128, NH], dt.float32, tag="A",
                               name=f"psA{i}"),
                 psb_pool.tile([128, NH], dt.float32, tag="B",
                               name=f"psB{i}"))
                for i in range(W)
            ]
            t1 = [t_stts(m) for m in range(W)]
            for k in range(KT):
                for m in range(W):
                    k_mms(m, k, *ps1[m])
                    if k == KT - 1:
                        epilogue(m, *ps1[m], t1[m])

            # ---- phase 2: m-outer, k-inner. The output DMA is split so
            # only a tiny [128,1,8] piece remains on the tail's critical
            # path; the first two pieces ride the idle scalar ring and
            # complete during compute. ----
            for m in range(W, MT):
                psA = psa_pool.tile([128, NH], dt.float32, tag="A",
                                    name=f"psA{m}")
                psB = psb_pool.tile([128, NH], dt.float32, tag="B",
                                    name=f"psB{m}")
                t_sb = t_stts(m)
                if m == MT - 1:
                    # last m-tile: A-half sweep first, its s-stt runs
                    # while the PE streams the B half -> shorter tail.
                    s_sb = epi.tile([128, NCENT], dt.float32, tag="s",
                                    name=f"s{m}")
                    for k in range(KT):
                        nc.tensor.matmul(
                            psA[:], ft_tiles[m][:, k, :],
                            ct_slice(k)[:, 0:NH],
                            start=(k == 0), stop=(k == KT - 1))
                    nc.vector.scalar_tensor_tensor(
                        out=s_sb[:, 0:NH], in0=psA[:], scalar=1.0,
                        in1=t_sb[:, 0:NH],
                        op0=mybir.AluOpType.mult,
                        op1=mybir.AluOpType.add)
                    for k in range(KT):
                        nc.tensor.matmul(
                            psB[:], ft_tiles[m][:, k, :],
                            ct_slice(k)[:, NH:NCENT],
                            start=(k == 0), stop=(k == KT - 1))
                    nc.vector.scalar_tensor_tensor(
                        out=s_sb[:, NH:NCENT], in0=psB[:], scalar=1.0,
                        in1=t_sb[:, NH:NCENT],
                        op0=mybir.AluOpType.mult,
                        op1=mybir.AluOpType.add)
                    mx_sb = epi.tile([128, 8], dt.float32, tag="mx",
                                     name=f"mx{m}")
                    nc.vector.max(mx_sb[:], s_sb[:])
                    nc.vector.max_index(stage[:, m, :], mx_sb[:], s_sb[:])
                else:
                    for k in range(KT):
                        k_mms(m, k, psA, psB)
                    epilogue(m, psA, psB, t_sb)
                if m == 7:
                    nc.scalar.dma_start(outp.ap()[:, 0:8, :],
                                        stage[:, 0:8, :])
                elif m == MT - 2:
                    nc.scalar.dma_start(outp.ap()[:, 8:MT - 1, :],
                                        stage[:, 8:MT - 1, :])

            nc.scalar.dma_start(outp.ap()[:, MT - 1:MT, :],
                                stage[:, MT - 1:MT, :])

    nc.compile()
    return nc


def _prep_inputs(feats, initc):
    dtf = _np_dtf()
    feats = np.ascontiguousarray(np.asarray(feats, dtype=np.float32))
    initc = np.ascontiguousarray(np.asarray(initc, dtype=np.float32))

    r = np.einsum("nd,nd->n", feats, feats)
    inv2 = (2.0 / np.sqrt(r + 1.0)).astype(np.float32)
    fn = (feats * inv2[:, None]).astype(dtf)

    ctm = np.ascontiguousarray(
        initc[:, :D].T.reshape(KT, 128, NCENT).transpose(1, 0, 2)
    ).astype(dtf)  # [128, KT, NCENT]
    cb = initc[:, D].astype(np.float32)
    hvv = np.einsum("kd,kd->k", initc, initc).astype(np.float32)
    cbb = np.ascontiguousarray(
        np.broadcast_to(cb[None, :], (128, NCENT)))
    hbb = np.ascontiguousarray(
        np.broadcast_to(hvv[None, :], (128, NCENT)))

    in_maps = []
    for c in range(NCORES):
        fc = fn[c * R:(c + 1) * R]  # [R, D]
        # X[m, p, k, j] = fc[m*128 + j, k*128 + p]
        X = np.ascontiguousarray(
            fc.reshape(MT, 128, KT, 128).transpose(0, 3, 2, 1)
        )
        # ivc[p, m] = inv2[c*R + m*128 + p]
        iv = np.ascontiguousarray(
            inv2[c * R:(c + 1) * R].reshape(MT, 128).T
        )
        in_maps.append({"ft": X, "ct": ctm, "cbb": cbb, "hbb": hbb,
                        "ivc": iv})
    return in_maps


def _run(feats, initc, labelset, trace=False):
    from concourse.bass_utils import run_bass_kernel_spmd

    if "nc" not in _cache:
        _cache["nc"] = _build()
    nc = _cache["nc"]

    in_maps = _prep_inputs(feats, initc)
    res = run_bass_kernel_spmd(
        nc, in_maps, core_ids=list(range(NCORES)), trace=trace
    )

    preds = np.concatenate(
        [
            res.results[c]["pred"][:, :, 0].T.reshape(R)
            for c in range(NCORES)
        ]
    ).astype(np.int64)
    labelset = np.asarray(labelset)
    out = labelset[preds]
    return out, res


def kernel(feats, initc, labelset):
    out, _ = _run(feats, initc, labelset, trace=False)
    return out


# revision 28
# speedup vs baseline: 1.0039x; 1.0039x over previous
"""Trainium2 Bass kernel for nn_CenterAwarePseudoModule (retrieval_knn).

Reference (per row i of feats, per centroid j):
    f_i   = [feats_i, 1] / ||[feats_i, 1]||
    d2_ij = ||f_i||^2 + ||c_j||^2 - 2 f_i . c_j
    out_i = labelset[argmin_j sqrt(max(d2_ij, 0))]

argmin_j d2_ij == argmax_j s_ij with
    s_ij = G'_ij + (inv2_i * cb_j - h_j)
where inv2_i = 2 / sqrt(||feats_i||^2 + 1) (HOST-computed),
G' = (feats * inv2) @ initc[:, :D].T (feats pre-scaled on host),
cb_j = initc[j, D], h_j = ||c_j||^2. Per-row positive affine transforms
preserve the argmin; sqrt/clamp are monotone.

Structure (per core; rows data-parallel over 8 cores):
  - G' on the PE: stationary = ft k-tile [128, 128 rows] fp16, moving =
    ct k-tile halves [128, 500] fp16. fp16 keeps DMA at 2 B/elem (the
    prologue is DMA-bound) and its 11-bit mantissa keeps the argmax
    exact on this data (host-sim: 0 flips; bf16 had 1 flip -> rel err
    1.8e-2, too close to the 2e-2 gate; fp32 doubles prologue bytes).
  - bias entirely on DVE in fp32: t = cbb*inv2_m - hb precomputed per
    m-tile during the PE sweep (scalar_tensor_tensor, [128,1] scalar),
    epilogue s = psum + t, then top-8 max + max_index into a
    [128, MT, 8] staging tile; ONE output DMA at the end (16 scattered
    [128,1] column DMAs cost ~7us of completion-wait in the tail).
  - DMA issue is the ramp bottleneck: each DMA_DIRECT2D costs ~0.8us
    of serial engine time and nothing issues before a ~6.6us program
    preamble. So issue on BOTH hardware-DGE engines in parallel:
    ct + consts on the (otherwise idle) scalar engine, ft on sync.
  - ramp: first 4 m-tiles run k-outer (4 m x 2 cent-halves = 8 PSUM
    banks) so PE consumption tracks the ct stream; everything is
    SBUF-resident in fp16 (ct 32KB + ft 64KB per partition).
"""
import sys

sys.path.insert(0, "/opt/trn_rl_repo")

import numpy as np

N, D, NCENT = 16384, 2048, 1000
NCORES = 8
R = N // NCORES          # rows per core = 2048
MT = R // 128            # m-tiles per core = 16
KT = D // 128            # contraction tiles = 16
NH = 500                 # centroid half (fits one PSUM bank: 500 fp32)
W = 4                    # phase-1 m-tile window (W*2 PSUM banks)

DTF_NAME = "float16"     # PE input dtype: float16 | bfloat16 | float32r

_cache = {}


def _np_dtf():
    if DTF_NAME == "float16":
        return np.float16
    if DTF_NAME == "bfloat16":
        import ml_dtypes
        return ml_dtypes.bfloat16
    return np.float32


def _build():
    import concourse.bacc as bacc
    import concourse.tile as tile
    from concourse import mybir

    dt = mybir.dt
    dtf = getattr(dt, DTF_NAME)

    nc = bacc.Bacc("TRN2", target_bir_lowering=False, debug=False)

    ft = nc.dram_tensor("ft", [MT, 128, KT, 128], dtf, kind="ExternalInput")
    ct = nc.dram_tensor("ct", [128, KT, NCENT], dtf, kind="ExternalInput")
    cbb = nc.dram_tensor("cbb", [128, NCENT], dt.float32, kind="ExternalInput")
    hbb = nc.dram_tensor("hbb", [128, NCENT], dt.float32, kind="ExternalInput")
    ivc = nc.dram_tensor("ivc", [128, MT], dt.float32, kind="ExternalInput")
    outp = nc.dram_tensor("pred", [128, MT, 8], dt.uint32, kind="ExternalOutput")

    with tile.TileContext(nc) as tc:
        with (
            tc.tile_pool(name="const", bufs=1) as constp,
            tc.tile_pool(name="epi", bufs=2) as epi,
            tc.tile_pool(name="tp", bufs=5) as tpool,
            tc.tile_pool(name="psA", bufs=W, space="PSUM") as psa_pool,
            tc.tile_pool(name="psB", bufs=W, space="PSUM") as psb_pool,
        ):
            ivc_sb = constp.tile([128, MT], dt.float32, tag="ivc")
            cbb_sb = constp.tile([128, NCENT], dt.float32, tag="cbb")
            hb_sb = constp.tile([128, NCENT], dt.float32, tag="hb")
            stage = constp.tile([128, MT, 8], dt.uint32, tag="stage")

            # ct lives in 5 grouped tiles so each can land in ONE big
            # DMA; ct_slice(k) picks the k-tile view inside its group.
            CT_GROUPS = [(0, 1), (1, 4), (4, 8), (8, 12), (12, 16)]
            ct_group_tiles = [
                constp.tile([128, hi - lo, NCENT], dtf, tag=f"ctg{lo}",
                            name=f"ctg{lo}")
                for lo, hi in CT_GROUPS
            ]
            ct_group_of = {}
            for gi, (lo, hi) in enumerate(CT_GROUPS):
                for k in range(lo, hi):
                    ct_group_of[k] = (gi, lo)

            def ct_slice(k):
                gi, lo = ct_group_of[k]
                return ct_group_tiles[gi][:, k - lo, :]
            ft_tiles = [
                constp.tile([128, KT, 128], dtf, tag=f"ft{m}", name=f"ftt{m}")
                for m in range(MT)
            ]

            # ---- DMA plan. One ring (parallel rings do NOT add
            # bandwidth: v4 measured 230+115 B/ns split vs 405 single).
            # The ring holds ~8 DMA instructions and recycles their
            # completion semaphores, so many small DMAs throttle on the
            # ~1.5us completion-notification lag, not on bandwidth ->
            # use FEW, BIG DMAs in arrival-priority order. ivc rides
            # the scalar ring (tiny, needed only by the DVE bias). ----
            nc.scalar.dma_start(ivc_sb[:], ivc.ap())

            H = KT // 2

            def dma_ct_group(lo, hi):
                gi = [g for g, (l, h) in enumerate(CT_GROUPS)
                      if l == lo and h == hi][0]
                nc.sync.dma_start(
                    ct_group_tiles[gi][:], ct.ap()[:, lo:hi, :]
                )

            def dma_ft_half(m, half):
                lo, hi = (0, H) if half == 0 else (H, KT)
                nc.sync.dma_start(
                    ft_tiles[m][:, lo:hi, :], ft.ap()[m][:, lo:hi, :]
                )

            # Arrival order matches the phase-1 emission (m-major
            # half-sweeps): m0's first half-sweep needs only ctg0/1/2 +
            # f00, so the PE runs gap-free from the first matmul and
            # the HAM clock-gate warms ~15us in; later items land well
            # before their m-runs start.
            dma_ct_group(0, 1)
            dma_ft_half(0, 0)
            dma_ct_group(1, 4)
            dma_ct_group(4, 8)
            dma_ft_half(1, 0)
            dma_ft_half(2, 0)
            dma_ft_half(3, 0)
            dma_ct_group(8, 12)
            dma_ct_group(12, 16)
            dma_ft_half(0, 1)
            dma_ft_half(1, 1)
            dma_ft_half(2, 1)
            dma_ft_half(3, 1)
            nc.sync.dma_start(cbb_sb[:], cbb.ap())
            nc.sync.dma_start(hb_sb[:], hbb.ap())
            for m in range(W, MT):
                nc.sync.dma_start(ft_tiles[m][:], ft.ap()[m])

            def k_mms(m, k, psA, psB):
                lhs = ft_tiles[m][:, k, :]
                ctk = ct_slice(k)
                first = k == 0
                last = k == KT - 1
                nc.tensor.matmul(psA[:], lhs, ctk[:, 0:NH],
                                 start=first, stop=last)
                nc.tensor.matmul(psB[:], lhs, ctk[:, NH:NCENT],
                                 start=first, stop=last)

            def t_stts(m):
                # t = cbb * inv2_m - hb   (DVE, runs during the PE sweep)
                t_sb = tpool.tile([128, NCENT], dt.float32, tag="t",
                                  name=f"t{m}")
                for lo, hi in ((0, NH), (NH, NCENT)):
                    nc.vector.scalar_tensor_tensor(
                        out=t_sb[:, lo:hi], in0=cbb_sb[:, lo:hi],
                        scalar=ivc_sb[:, m:m + 1], in1=hb_sb[:, lo:hi],
                        op0=mybir.AluOpType.mult,
                        op1=mybir.AluOpType.subtract,
                    )
                return t_sb

            def epilogue(m, psA, psB, t_sb):
                s_sb = epi.tile([128, NCENT], dt.float32, tag="s",
                                name=f"s{m}")
                nc.vector.scalar_tensor_tensor(
                    out=s_sb[:, 0:NH], in0=psA[:], scalar=1.0,
                    in1=t_sb[:, 0:NH],
                    op0=mybir.AluOpType.mult, op1=mybir.AluOpType.add,
                )
                nc.vector.scalar_tensor_tensor(
                    out=s_sb[:, NH:NCENT], in0=psB[:], scalar=1.0,
                    in1=t_sb[:, NH:NCENT],
                    op0=mybir.AluOpType.mult, op1=mybir.AluOpType.add,
                )
                mx_sb = epi.tile([128, 8], dt.float32, tag="mx",
                                 name=f"mx{m}")
                nc.vector.max(mx_sb[:], s_sb[:])
                nc.vector.max_index(stage[:, m, :], mx_sb[:], s_sb[:])

            # ---- phase 1: junk fp16 warm-up matmuls (the PE HAM
            # clock-gate starts at 1.2 GHz and needs ~3.4us of sustained
            # activity to reach 2.4 GHz; these run while the first ct/ft
            # DMAs stream in, and the real start=True k0 matmuls
            # overwrite the banks), then a k-outer sweep over the first
            # W m-tiles ----
            ps1 = [
                (psa_pool.tile([128, NH], dt.float32, tag="A",
                               name=f"psA{i}"),
                 psb_pool.tile([128, NH], dt.float32, tag="B",
                               name=f"psB{i}"))
                for i in range(W)
            ]
            t1 = [t_stts(m) for m in range(W)]
            for half in range(2):
                for m in range(W):
                    for k in range(half * H, half * H + H):
                        k_mms(m, k, *ps1[m])
                    if half == 1:
                        epilogue(m, *ps1[m], t1[m])

            # ---- phase 2: m-outer, k-inner. The output DMA is split so
            # only a tiny [128,1,8] piece remains on the tail's critical
            # path; the first two pieces ride the idle scalar ring and
            # complete during compute. ----
            for m in range(W, MT):
                psA = psa_pool.tile([128, NH], dt.float32, tag="A",
                                    name=f"psA{m}")
                psB = psb_pool.tile([128, NH], dt.float32, tag="B",
                                    name=f"psB{m}")
                t_sb = t_stts(m)
                if m == MT - 1:
                    # last m-tile: A-half sweep first, its s-stt runs
                    # while the PE streams the B half -> shorter tail.
                    s_sb = epi.tile([128, NCENT], dt.float32, tag="s",
                                    name=f"s{m}")
                    for k in range(KT):
                        nc.tensor.matmul(
                            psA[:], ft_tiles[m][:, k, :],
                            ct_slice(k)[:, 0:NH],
                            start=(k == 0), stop=(k == KT - 1))
                    nc.vector.scalar_tensor_tensor(
                        out=s_sb[:, 0:NH], in0=psA[:], scalar=1.0,
                        in1=t_sb[:, 0:NH],
                        op0=mybir.AluOpType.mult,
                        op1=mybir.AluOpType.add)
                    for k in range(KT):
                        nc.tensor.matmul(
                            psB[:], ft_tiles[m][:, k, :],
                            ct_slice(k)[:, NH:NCENT],
                            start=(k == 0), stop=(k == KT - 1))
                    nc.vector.scalar_tensor_tensor(
                        out=s_sb[:, NH:NCENT], in0=psB[:], scalar=1.0,
                        in1=t_sb[:, NH:NCENT],
                        op0=mybir.AluOpType.mult,
                        op1=mybir.AluOpType.add)
                    mx_sb = epi.tile([128, 8], dt.float32, tag="mx",
                                     name=f"mx{m}")
                    nc.vector.max(mx_sb[:], s_sb[:])
                    nc.vector.max_index(stage[:, m, :], mx_sb[:], s_sb[:])
                else:
                    for k in range(KT):
                        k_mms(m, k, psA, psB)
                    epilogue(m, psA, psB, t_sb)
                if m == 7:
                    nc.scalar.dma_start(outp.ap()[:, 0:8, :],
                                        stage[:, 0:8, :])
                elif m == MT - 2:
                    nc.scalar.dma_start(outp.ap()[:, 8:MT - 1, :],
                                        stage[:, 8:MT - 1, :])

            nc.scalar.dma_start(outp.ap()[:, MT - 1:MT, :],
                                stage[:, MT - 1:MT, :])

    nc.compile()
    return nc


def _prep_inputs(feats, initc):
    dtf = _np_dtf()
    feats = np.ascontiguousarray(np.asarray(feats, dtype=np.float32))
    initc = np.ascontiguousarray(np.asarray(initc, dtype=np.float32))

    r = np.einsum("nd,nd->n", feats, feats)
    inv2 = (2.0 / np.sqrt(r + 1.0)).astype(np.float32)
    fn = (feats * inv2[:, None]).astype(dtf)

    ctm = np.ascontiguousarray(
        initc[:, :D].T.reshape(KT, 128, NCENT).transpose(1, 0, 2)
    ).astype(dtf)  # [128, KT, NCENT]
    cb = initc[:, D].astype(np.float32)
    hvv = np.einsum("kd,kd->k", initc, initc).astype(np.float32)
    cbb = np.ascontiguousarray(
        np.broadcast_to(cb[None, :], (128, NCENT)))
    hbb = np.ascontiguousarray(
        np.broadcast_to(hvv[None, :], (128, NCENT)))

    in_maps = []
    for c in range(NCORES):
        fc = fn[c * R:(c + 1) * R]  # [R, D]
        # X[m, p, k, j] = fc[m*128 + j, k*128 + p]
        X = np.ascontiguousarray(
            fc.reshape(MT, 128, KT, 128).transpose(0, 3, 2, 1)
        )
        # ivc[p, m] = inv2[c*R + m*128 + p]
        iv = np.ascontiguousarray(
            inv2[c * R:(c + 1) * R].reshape(MT, 128).T
        )
        in_maps.append({"ft": X, "ct": ctm, "cbb": cbb, "hbb": hbb,
                        "ivc": iv})
    return in_maps


def _run(feats, initc, labelset, trace=False):
    from concourse.bass_utils import run_bass_kernel_spmd

    if "nc" not in _cache:
        _cache["nc"] = _build()
    nc = _cache["nc"]

    in_maps = _prep_inputs(feats, initc)
    res = run_bass_kernel_spmd(
        nc, in_maps, core_ids=list(range(NCORES)), trace=trace
    )

    preds = np.concatenate(
        [
            res.results[c]["pred"][:, :, 0].T.reshape(R)
            for c in range(NCORES)
        ]
    ).astype(np.int64)
    labelset = np.asarray(labelset)
    out = labelset[preds]
    return out, res


def kernel(feats, initc, labelset):
    out, _ = _run(feats, initc, labelset, trace=False)
    return out
